# revision 10
# baseline (speedup 1.0000x reference)
"""Trainium2 Bass kernel for nn_CBSA_9517647528038 (sparse landmark attention).

Math (per batch sample b, head h, dh=64, 784 patches + 1 cls token):
  inner   = x @ W_in                                  [785, 768] -> 12 heads x 64
  reps    = Pfull @ patches            (adaptive pool) [64, 64]
  attn_u  = exp(SCALE * reps @ patches^T)             [64, 784]  (unnormalized)
  reps'   = reps + step_rep * (attn_u @ patches) / rowsum(attn_u)
  attn2_u = exp(SCALE * reps' @ reps'^T)              [64, 64]   (symmetric!)
  rtr     = attn2_u @ reps' / rowsum(attn2_u)
  deltaT  = (step_x/rowsum(attn_u) * rtr)^T-contracted with attn_u  [64d, 784]
  y       = x + concat(cls, delta) @ W_out

Distribution: data-parallel over batch, 8 samples per core, 8 cores, no
collectives.

Implementation notes:
  * All transposes are done as normal-mode matmuls streaming a fp16
    identity (out = data^T @ I). Transpose-mode matmuls don't count as
    PE-busy for the HAM clock gate and run ~3x slower in practice.
  * patches in natural [token, hd] layout (pnat) are derived from the
    transposed projection inT by 42 such identity matmuls per sample
    instead of recomputing the projection (84 f32r matmuls).
  * Pair-packed heads (2 per 128 partitions).  Logits and delta_tokens
    use block-diagonal stationaries so one 784-column stream serves both
    heads.  Pooling duplicates the pool matrix columns so one stream
    yields both the aligned and shifted landmark stacks.
  * Emission is software-pipelined: engine queues are FIFO, so the
    latency-sensitive attention chain of sample s is interleaved with
    filler matmul chunks (outproj of s-1, projection/pooling of s+1).
  * Dtypes: projections f32r (11-bit mantissa, full PE rate at N>=256),
    attention internals fp16.  Softmax max-subtraction is skipped
    (logits are O(0.3)); all normalizations fold into per-partition
    scale vectors.
"""

import numpy as np

HEADS = 12
DH = 64
REP = 8
GRID = 28
SCALE = DH ** -0.5
B, N, D = 64, 785, 768

NP = 786        # token dim padded to even (f32r matmul needs even N)
NC = 8          # cores
BS = B // NC    # samples per core
NPATCH = 784
NPAD = 896      # 7 * 128, zero padded patch dim
KT = 6          # 768 / 128 contraction tiles
NT = 7          # patch tiles of 128 (last has 16 valid rows)

_STATE = {}
SECTIONS = []  # (instruction-number, label) marks recorded during build


def _mark(nc, label):
    nm = nc.get_next_instruction_name()
    SECTIONS.append((int(nm.split("-")[-1]), label))


def _pool_matrix(in_size, out_size):
    P = np.zeros((out_size, in_size), np.float32)
    for i in range(out_size):
        s = (i * in_size) // out_size
        e = -((-(i + 1) * in_size) // out_size)
        P[i, s:e] = 1.0 / (e - s)
    return P


def _build_program():
    import concourse.bass as bass
    import concourse.tile as tile
    import concourse.mybir as mybir
    from concourse import bacc

    f32 = mybir.dt.float32
    f32r = mybir.dt.float32r
    f16 = mybir.dt.float16
    AF = mybir.ActivationFunctionType

    nc = bacc.Bacc(trn_type="TRN2", target_bir_lowering=False, debug=False)

    # ---- DRAM I/O (per core) ----
    xT_d = nc.dram_tensor("xT", [BS, D, NP], f32r, kind="ExternalInput")
    win_d = nc.dram_tensor("win", [D, D], f32r, kind="ExternalInput")
    wout_d = nc.dram_tensor("wout", [D, D], f16, kind="ExternalInput")
    pft2_d = nc.dram_tensor("pft2", [NPAD, 128], f16, kind="ExternalInput")
    idh_d = nc.dram_tensor("idh", [128, 128], f16, kind="ExternalInput")
    srp_d = nc.dram_tensor("srp", [128, 6], f32, kind="ExternalInput")
    sxp_d = nc.dram_tensor("sxp", [128, 6], f32, kind="ExternalInput")
    yT_d = nc.dram_tensor("yT", [BS, D, NP], f32, kind="ExternalOutput")

    with tile.TileContext(nc) as tc:
        with (
            tc.tile_pool(name="const", bufs=1) as pc,
            tc.tile_pool(name="sample", bufs=2) as psmp,
            tc.tile_pool(name="pair", bufs=3) as ppr,
            tc.tile_pool(name="stage", bufs=3) as pst,
            tc.tile_pool(name="pj1", bufs=2, space="PSUM") as pj1,
            tc.tile_pool(name="pj2", bufs=2, space="PSUM") as pj2,
            tc.tile_pool(name="ptp", bufs=2, space="PSUM") as ptp,
            tc.tile_pool(name="psm", bufs=2, space="PSUM") as psm,
        ):
            # ---- constants (win + sample-0 x first; the rest after) ----
            win_sb = [pc.tile([128, D], f32r, tag=f"win{k}", name=f"win{k}")
                      for k in range(KT)]
            wout_sb = [pc.tile([128, D], f16, tag=f"wout{k}", name=f"wout{k}")
                       for k in range(KT)]
            pft2_sb = [pc.tile([128, 128], f16, tag=f"pft{t}", name=f"pft{t}")
                       for t in range(NT)]
            idh = pc.tile([128, 128], f16, tag="idh", name="idh")
            srp = pc.tile([128, 6], f32, tag="srp", name="srp")
            sxp = pc.tile([128, 6], f32, tag="sxp", name="sxp")

            sample_xts = {}

            def emit_xts_dma(s):
                xts = [psmp.tile([128, NP], f32r, tag=f"xt{k}", name=f"xt{k}",
                                 bufs=3) for k in range(KT)]
                for k in range(KT):
                    nc.sync.dma_start(xts[k][:], xT_d[s, k * 128:(k + 1) * 128, :])
                sample_xts[s] = xts

            xts0 = [psmp.tile([128, NP], f32r, tag=f"xt{k}", name=f"xt{k}",
                              bufs=3) for k in range(KT)]
            for k in range(KT):
                nc.sync.dma_start(win_sb[k][:], win_d[k * 128:(k + 1) * 128, :])
                nc.sync.dma_start(xts0[k][:], xT_d[0, k * 128:(k + 1) * 128, :])
            sample_xts[0] = xts0
            nc.sync.dma_start(idh[:], idh_d[:])
            for t in range(NT):
                nc.sync.dma_start(pft2_sb[t][:], pft2_d[t * 128:(t + 1) * 128, :])
            for k in range(KT):
                nc.sync.dma_start(wout_sb[k][:], wout_d[k * 128:(k + 1) * 128, :])
            nc.sync.dma_start(srp[:], srp_d[:])
            nc.sync.dma_start(sxp[:], sxp_d[:])

            cp = [0]  # alternating PSUM-evac engine (ACT / DVE)

            def evac(dst, src):
                cp[0] += 1
                if cp[0] % 2:
                    nc.scalar.copy(dst, src)
                else:
                    nc.vector.tensor_copy(dst, src)

            # per-sample persistent tiles
            sample_tiles = {}

            def get_sample(s):
                if s not in sample_tiles:
                    sample_tiles[s] = {
                        "inT": [psmp.tile([128, NP], f16, tag=f"inT{m}",
                                          name=f"inT{m}") for m in range(KT)],
                        "pnat": [psmp.tile([128, D], f16, tag=f"pn{t}",
                                           name=f"pn{t}") for t in range(NT)],
                        "rstk": psmp.tile([128, D], f16, tag="rstk", name="rstk"),
                        "dT": [psmp.tile([128, NP], f16, tag=f"dT{m}",
                                         name=f"dT{m}") for m in range(KT)],
                    }
                return sample_tiles[s]

            # ---------------- filler thunk builders ----------------

            def innerT_chunk(s, m, c0, c1):
                # inT[m][:, c0:c1] = (W_in^T x^T)[m-block, cols], fp16
                def th():
                    _mark(nc, f"inner{s}")
                    st = get_sample(s)
                    xts = sample_xts[s]
                    ps = pj1.tile([128, 512], f32, tag="proj", name="proj")
                    for k in range(KT):
                        nc.tensor.matmul(
                            ps[:, 0:c1 - c0],
                            win_sb[k][:, m * 128:(m + 1) * 128],
                            xts[k][:, c0:c1],
                            start=(k == 0), stop=(k == KT - 1),
                        )
                    evac(st["inT"][m][:, c0:c1], ps[:, 0:c1 - c0])
                return th

            def pnatT_group(s, t):
                # pnat[t] = transpose of inT token-block t (identity matmuls)
                def th():
                    _mark(nc, f"pnatT{s}")
                    st = get_sample(s)
                    pn = st["pnat"][t]
                    sz = 128 if t < NT - 1 else 16
                    if t == NT - 1:
                        nc.gpsimd.memset(pn[:], 0.0)
                    tp1 = ptp.tile([128, 512], f32, tag="tp", name="tp1")
                    tp2 = ptp.tile([128, 512], f32, tag="tp", name="tp2")
                    for hp in range(KT):
                        dst = tp1[0:sz, (hp % 4) * 128:(hp % 4) * 128 + 128] \
                            if hp < 4 else \
                            tp2[0:sz, (hp - 4) * 128:(hp - 4) * 128 + 128]
                        nc.tensor.matmul(
                            dst, st["inT"][hp][:, 1 + t * 128:1 + t * 128 + sz],
                            idh[:], start=True, stop=True,
                        )
                    evac(pn[0:sz, 0:512], tp1[0:sz, 0:512])
                    evac(pn[0:sz, 512:768], tp2[0:sz, 0:256])
                return th

            def pool_chunk(s, ci):
                # pooled landmark stack; stationary has duplicated columns so
                # rows 0:64 and 64:128 both hold the pooled matrix
                def th():
                    _mark(nc, f"pool{s}")
                    st = get_sample(s)
                    rstk = st["rstk"]
                    ps = pj1.tile([128, 512], f32, tag="proj", name="proj")
                    c0, c1 = (0, 512) if ci == 0 else (512, 768)
                    for t in range(NT):
                        nc.tensor.matmul(
                            ps[:, 0:c1 - c0], pft2_sb[t][:],
                            st["pnat"][t][:, c0:c1],
                            start=(t == 0), stop=(t == NT - 1),
                        )
                    if ci == 0:
                        evac(rstk[0:64, 0:512], ps[0:64, 0:512])
                        evac(rstk[64:128, 0:448], ps[64:128, 64:512])
                    else:
                        evac(rstk[0:64, 512:768], ps[0:64, 0:256])
                        evac(rstk[64:128, 448:704], ps[64:128, 0:256])
                return th

            def outproj_chunk(s, m, c0, c1):
                def th():
                    _mark(nc, f"out{s}")
                    st = get_sample(s)
                    xts = sample_xts[s]
                    ps = pj1.tile([128, 512], f32, tag="proj", name="proj")
                    for k in range(KT):
                        nc.tensor.matmul(
                            ps[:, 0:c1 - c0],
                            wout_sb[k][:, m * 128:(m + 1) * 128],
                            st["dT"][k][:, c0:c1],
                            start=(k == 0), stop=(k == KT - 1),
                        )
                    yst = pst.tile([128, 512], f32, tag="yst", name="yst")
                    xsl = xts[m][:, c0:c1].bitcast(f32)
                    cp[0] += 1
                    if cp[0] % 2:
                        nc.vector.tensor_add(
                            yst[:, 0:c1 - c0], ps[:, 0:c1 - c0], xsl)
                    else:
                        nc.scalar.copy(yst[:, 0:c1 - c0], ps[:, 0:c1 - c0])
                        nc.gpsimd.tensor_add(
                            yst[:, 0:c1 - c0], yst[:, 0:c1 - c0], xsl)
                    nc.sync.dma_start(
                        yT_d[s, m * 128:(m + 1) * 128, c0:c1],
                        yst[:, 0:c1 - c0])
                return th

            def prologue_thunks(s):
                th = []
                for m in range(KT):
                    for c0, c1 in ((0, 512), (512, NP)):
                        th.append(innerT_chunk(s, m, c0, c1))
                for t in range(NT):
                    th.append(pnatT_group(s, t))
                th.append(pool_chunk(s, 0))
                th.append(pool_chunk(s, 1))
                return th

            def outproj_thunks(s):
                return [outproj_chunk(s, m, c0, c1)
                        for m in range(KT) for c0, c1 in ((0, 512), (512, NP))]

            # ---------------- the attention pair chain ----------------

            def emit_pairs(s, filler):
                st = get_sample(s)
                rstk, inT, pnat, dT = (st["rstk"], st["inT"], st["pnat"],
                                       st["dT"])

                def fill():
                    if filler:
                        filler.pop(0)()

                for hp in range(KT):
                    col = hp * 128  # reps col offset (both rstk halves)

                    # scratch psum: one bank, sliced per stage
                    sm = psm.tile([128, 512], f32, tag="sm", name="sm")

                    _mark(nc, f"p{s}.G1")
                    # G1: repsT via identity matmuls -> block-diag stationary
                    nc.tensor.matmul(sm[0:64, 0:64], rstk[0:64, col:col + 64],
                                     idh[0:64, 0:64], start=True, stop=True)
                    nc.tensor.matmul(sm[64:128, 64:128],
                                     rstk[64:128, col:col + 64],
                                     idh[64:128, 64:128], start=True, stop=True)
                    bd1 = ppr.tile([128, 128], f16, tag="bd1", name="bd1")
                    nc.gpsimd.memset(bd1[:], 0.0)
                    nc.scalar.copy(bd1[0:64, 0:64], sm[0:64, 0:64])
                    nc.vector.tensor_copy(bd1[64:128, 64:128],
                                          sm[64:128, 64:128])
                    fill()

                    _mark(nc, f"p{s}.G2")
                    # G2: logits for both heads in one stream (block-diag)
                    au = ppr.tile([128, NPAD], f16, tag="au", name="au")
                    nc.gpsimd.memset(au[:, NPATCH:NPAD], 0.0)
                    pl1 = pj2.tile([128, 512], f32, tag="proj", name="proj")
                    nc.tensor.matmul(pl1[:, 0:512], bd1[:], inT[hp][:, 1:513],
                                     start=True, stop=True)
                    pl2c = pj2.tile([128, 512], f32, tag="proj", name="proj")
                    nc.tensor.matmul(pl2c[:, 0:NPATCH - 512], bd1[:],
                                     inT[hp][:, 513:1 + NPATCH],
                                     start=True, stop=True)
                    nc.scalar.activation(au[:, 0:512], pl1[:, 0:512], AF.Exp,
                                         scale=SCALE)
                    nc.scalar.activation(au[:, 512:NPATCH],
                                         pl2c[:, 0:NPATCH - 512],
                                         AF.Exp, scale=SCALE)
                    den = ppr.tile([128, 1], f32, tag="den", name="den")
                    nc.vector.tensor_reduce(den[:], au[:, 0:NPATCH],
                                            mybir.AxisListType.XYZW,
                                            mybir.AluOpType.add)
                    rcp = ppr.tile([128, 1], f32, tag="rcp", name="rcp")
                    nc.vector.reciprocal(rcp[:], den[:])
                    fill()

                    _mark(nc, f"p{s}.G3")
                    # G3: attn_uT via identity matmuls, packed 4-to-a-bank
                    auT = ppr.tile([128, NPAD], f16, tag="auT", name="auT")
                    tp1 = ptp.tile([128, 512], f32, tag="tp", name="tp1")
                    tp2 = ptp.tile([128, 512], f32, tag="tp", name="tp2")
                    for t in range(NT):
                        dst = tp1[:, (t % 4) * 128:(t % 4) * 128 + 128] \
                            if t < 4 else tp2[:, (t - 4) * 128:(t - 4) * 128 + 128]
                        nc.tensor.matmul(dst, au[:, t * 128:(t + 1) * 128],
                                         idh[:], start=True, stop=True)
                    evac(auT[:, 0:512], tp1[:, 0:512])
                    evac(auT[:, 512:NPAD], tp2[:, 0:NPAD - 512])
                    fill()

                    _mark(nc, f"p{s}.G4")
                    # G4: rep_delta (pair-fused, diag blocks valid)
                    for t in range(NT):
                        nc.tensor.matmul(
                            sm[:, 128:256], auT[:, t * 128:(t + 1) * 128],
                            pnat[t][:, col:col + 128],
                            start=(t == 0), stop=(t == NT - 1),
                        )
                    svec = ppr.tile([128, 1], f32, tag="svec", name="svec")
                    nc.vector.tensor_scalar(
                        svec[:], rcp[:], srp[:, hp:hp + 1], None,
                        op0=mybir.AluOpType.mult)
                    ru = ppr.tile([128, DH], f16, tag="ru", name="ru")
                    nc.scalar.mul(ru[0:64, :], sm[0:64, 128:192], svec[0:64, :])
                    nc.scalar.mul(ru[64:128, :], sm[64:128, 192:256],
                                  svec[64:128, :])
                    nc.gpsimd.tensor_add(ru[0:64, :], ru[0:64, :],
                                         rstk[0:64, col:col + 64])
                    nc.gpsimd.tensor_add(ru[64:128, :], ru[64:128, :],
                                         rstk[64:128, col:col + 64])
                    fill()

                    _mark(nc, f"p{s}.G5")
                    # G5: reps'^T (identity matmuls), stacked layout
                    nc.tensor.matmul(sm[0:64, 256:320], ru[0:64, :],
                                     idh[0:64, 0:64], start=True, stop=True)
                    nc.tensor.matmul(sm[64:128, 320:384], ru[64:128, :],
                                     idh[64:128, 64:128], start=True, stop=True)
                    ruT = ppr.tile([128, DH], f16, tag="ruT", name="ruT")
                    nc.scalar.copy(ruT[0:64, :], sm[0:64, 256:320])
                    nc.vector.tensor_copy(ruT[64:128, :], sm[64:128, 320:384])
                    fill()

                    _mark(nc, f"p{s}.G6")
                    # G6: attn2_u = exp(SCALE * reps' @ reps'^T)  (symmetric)
                    nc.tensor.matmul(sm[0:64, 384:448], ruT[0:64, :],
                                     ruT[0:64, :], start=True, stop=True)
                    nc.tensor.matmul(sm[64:128, 384:448], ruT[64:128, :],
                                     ruT[64:128, :], start=True, stop=True)
                    a2 = ppr.tile([128, DH], f16, tag="a2", name="a2")
                    d3 = ppr.tile([128, 1], f32, tag="d3", name="d3")
                    nc.scalar.activation(a2[:], sm[:, 384:448], AF.Exp,
                                         scale=SCALE)
                    nc.vector.tensor_reduce(d3[:], a2[:],
                                            mybir.AxisListType.XYZW,
                                            mybir.AluOpType.add)
                    rcp2 = ppr.tile([128, 1], f32, tag="rcp2", name="rcp2")
                    nc.vector.reciprocal(rcp2[:], d3[:])
                    fill()

                    _mark(nc, f"p{s}.G7")
                    # G7: rtr = attn2_u @ reps' (a2 symmetric -> use as lhsT)
                    nc.tensor.matmul(sm[0:64, 448:512], a2[0:64, :],
                                     ru[0:64, :], start=True, stop=True)
                    nc.tensor.matmul(sm[64:128, 448:512], a2[64:128, :],
                                     ru[64:128, :], start=True, stop=True)
                    s2 = ppr.tile([128, 1], f32, tag="s2", name="s2")
                    nc.vector.tensor_scalar(
                        s2[:], rcp2[:], rcp[:], sxp[:, hp:hp + 1],
                        op0=mybir.AluOpType.mult, op1=mybir.AluOpType.mult)
                    bd2 = ppr.tile([128, 128], f16, tag="bd2", name="bd2")
                    nc.gpsimd.memset(bd2[:], 0.0)
                    nc.vector.tensor_scalar(
                        bd2[0:64, 0:64], sm[0:64, 448:512], s2[0:64, :], None,
                        op0=mybir.AluOpType.mult)
                    nc.vector.tensor_scalar(
                        bd2[64:128, 64:128], sm[64:128, 448:512],
                        s2[64:128, :], None, op0=mybir.AluOpType.mult)
                    fill()

                    _mark(nc, f"p{s}.G8")
                    # G8: delta_tokens^T for both heads (block-diag stationary)
                    pd1 = pj2.tile([128, 512], f32, tag="proj", name="proj")
                    nc.tensor.matmul(pd1[:, 0:512], bd2[:], au[:, 0:512],
                                     start=True, stop=True)
                    pd2 = pj2.tile([128, 512], f32, tag="proj", name="proj")
                    nc.tensor.matmul(pd2[:, 0:NPATCH - 512], bd2[:],
                                     au[:, 512:NPATCH], start=True, stop=True)
                    evac(dT[hp][:, 1:513], pd1[:, 0:512])
                    evac(dT[hp][:, 513:1 + NPATCH], pd2[:, 0:NPATCH - 512])
                    nc.vector.tensor_copy(dT[hp][:, 0:1], inT[hp][:, 0:1])
                    fill()

            # ---------------- pipelined emission ----------------

            for th in prologue_thunks(0):
                th()
            for s in range(BS):
                filler = []
                if s + 1 < BS:
                    filler.append(lambda s=s: emit_xts_dma(s + 1))
                    filler += prologue_thunks(s + 1)
                if s > 0:
                    filler += outproj_thunks(s - 1)
                emit_pairs(s, filler)
                for th in filler:
                    th()
                # free the sample-tile dict entry (tiles rotate via tags)
                sample_tiles.pop(s - 1, None)
                sample_xts.pop(s - 1, None)
            for th in outproj_thunks(BS - 1):
                th()

    nc.finalize()
    return nc


def _get_state():
    if "nc" not in _STATE:
        _STATE["nc"] = _build_program()
    return _STATE["nc"]


def _host_inputs(x, W_in, W_out, step_x, step_rep):
    Ph = _pool_matrix(GRID, REP)
    Pfull = np.kron(Ph, Ph)                      # [64, 784]
    pft2 = np.zeros((NPAD, 128), np.float16)
    pft2[:NPATCH, 0:64] = Pfull.T.astype(np.float16)
    pft2[:NPATCH, 64:128] = Pfull.T.astype(np.float16)

    srp = np.zeros((128, 6), np.float32)
    sxp = np.zeros((128, 6), np.float32)
    sr = np.asarray(step_rep).reshape(HEADS)
    sx = np.asarray(step_x).reshape(HEADS)
    for hp in range(6):
        srp[0:64, hp] = sr[2 * hp]
        srp[64:128, hp] = sr[2 * hp + 1]
        sxp[0:64, hp] = sx[2 * hp]
        sxp[64:128, hp] = sx[2 * hp + 1]

    com = {
        "win": np.ascontiguousarray(W_in, np.float32),
        "wout": np.ascontiguousarray(W_out).astype(np.float16),
        "pft2": pft2,
        "idh": np.eye(128, dtype=np.float16),
        "srp": srp,
        "sxp": sxp,
    }
    xT = np.zeros((B, D, NP), np.float32)
    xT[:, :, 0:N] = np.transpose(x, (0, 2, 1))
    in_maps = []
    for c in range(NC):
        m = dict(com)
        m["xT"] = xT[c * BS:(c + 1) * BS]
        in_maps.append(m)
    return in_maps


def kernel(x, W_in, W_out, step_x, step_rep):
    from concourse import bass2jax
    nc = _get_state()
    in_maps = _host_inputs(x, W_in, W_out, step_x, step_rep)
    y = np.empty((B, N, D), np.float32)
    # The axon-tunneled result fetch has been observed to intermittently
    # return corrupted (NaN) buffers while the on-device result is fine;
    # re-run on a non-finite result.
    for _ in range(3):
        res = bass2jax.run_bass_via_pjrt(nc, in_maps, NC)
        for c in range(NC):
            y[c * BS:(c + 1) * BS] = np.transpose(
                res[c]["yT"][:, :, 0:N], (0, 2, 1))
        if np.isfinite(y).all():
            break
    return y


# revision 11
# speedup vs baseline: 1.1257x; 1.1257x over previous
"""Trainium2 Bass kernel for nn_CBSA_9517647528038 (sparse landmark attention).

Math (per batch sample b, head h, dh=64, 784 patches + 1 cls token):
  inner   = x @ W_in                                  [785, 768] -> 12 heads x 64
  reps    = Pfull @ patches            (adaptive pool) [64, 64]
  attn_u  = exp(SCALE * reps @ patches^T)             [64, 784]  (unnormalized)
  reps'   = reps + step_rep * (attn_u @ patches) / rowsum(attn_u)
  attn2_u = exp(SCALE * reps' @ reps'^T)              [64, 64]   (symmetric!)
  rtr     = attn2_u @ reps' / rowsum(attn2_u)
  deltaT  = (step_x/rowsum(attn_u) * rtr)^T-contracted with attn_u  [64d, 784]
  y       = x + concat(cls, delta) @ W_out

Distribution: data-parallel over batch, 8 samples per core, 8 cores, no
collectives.

Implementation notes:
  * All transposes are done as normal-mode matmuls streaming a fp16
    identity (out = data^T @ I). Transpose-mode matmuls don't count as
    PE-busy for the HAM clock gate and run ~3x slower in practice.
  * patches in natural [token, hd] layout (pnat) are derived from the
    transposed projection inT by 42 such identity matmuls per sample
    instead of recomputing the projection (84 f32r matmuls).
  * Pair-packed heads (2 per 128 partitions).  Logits and delta_tokens
    use block-diagonal stationaries so one 784-column stream serves both
    heads.  Pooling duplicates the pool matrix columns so one stream
    yields both the aligned and shifted landmark stacks.
  * Emission is software-pipelined: engine queues are FIFO, so the
    latency-sensitive attention chain of sample s is interleaved with
    filler matmul chunks (outproj of s-1, projection/pooling of s+1).
  * Dtypes: projections f32r (11-bit mantissa, full PE rate at N>=256),
    attention internals fp16.  Softmax max-subtraction is skipped
    (logits are O(0.3)); all normalizations fold into per-partition
    scale vectors.
"""

import numpy as np

HEADS = 12
DH = 64
REP = 8
GRID = 28
SCALE = DH ** -0.5
B, N, D = 64, 785, 768

NP = 786        # token dim padded to even (f32r matmul needs even N)
NC = 8          # cores
BS = B // NC    # samples per core
NPATCH = 784
NPAD = 896      # 7 * 128, zero padded patch dim
KT = 6          # 768 / 128 contraction tiles
NT = 7          # patch tiles of 128 (last has 16 valid rows)

_STATE = {}
SECTIONS = []  # (instruction-number, label) marks recorded during build


def _mark(nc, label):
    nm = nc.get_next_instruction_name()
    SECTIONS.append((int(nm.split("-")[-1]), label))


def _pool_matrix(in_size, out_size):
    P = np.zeros((out_size, in_size), np.float32)
    for i in range(out_size):
        s = (i * in_size) // out_size
        e = -((-(i + 1) * in_size) // out_size)
        P[i, s:e] = 1.0 / (e - s)
    return P


def _build_program():
    import concourse.bass as bass
    import concourse.tile as tile
    import concourse.mybir as mybir
    from concourse import bacc

    f32 = mybir.dt.float32
    f32r = mybir.dt.float32r
    f16 = mybir.dt.float16
    AF = mybir.ActivationFunctionType

    nc = bacc.Bacc(trn_type="TRN2", target_bir_lowering=False, debug=False)

    # ---- DRAM I/O (per core) ----
    xT_d = nc.dram_tensor("xT", [BS, D, NP], f32r, kind="ExternalInput")
    win_d = nc.dram_tensor("win", [D, D], f32r, kind="ExternalInput")
    wout_d = nc.dram_tensor("wout", [D, D], f16, kind="ExternalInput")
    pft2_d = nc.dram_tensor("pft2", [NPAD, 128], f16, kind="ExternalInput")
    idh_d = nc.dram_tensor("idh", [128, 128], f16, kind="ExternalInput")
    srp_d = nc.dram_tensor("srp", [128, 6], f32, kind="ExternalInput")
    sxp_d = nc.dram_tensor("sxp", [128, 6], f32, kind="ExternalInput")
    yT_d = nc.dram_tensor("yT", [BS, D, NP], f32, kind="ExternalOutput")

    with tile.TileContext(nc) as tc:
        with (
            tc.tile_pool(name="const", bufs=1) as pc,
            tc.tile_pool(name="sample", bufs=2) as psmp,
            tc.tile_pool(name="pair", bufs=3) as ppr,
            tc.tile_pool(name="stage", bufs=3) as pst,
            tc.tile_pool(name="pj1", bufs=2, space="PSUM") as pj1,
            tc.tile_pool(name="pj2", bufs=2, space="PSUM") as pj2,
            tc.tile_pool(name="ptp", bufs=2, space="PSUM") as ptp,
            tc.tile_pool(name="psm", bufs=2, space="PSUM") as psm,
        ):
            # ---- constants (win + sample-0 x first; the rest after) ----
            win_sb = [pc.tile([128, D], f32r, tag=f"win{k}", name=f"win{k}")
                      for k in range(KT)]
            wout_sb = [pc.tile([128, D], f16, tag=f"wout{k}", name=f"wout{k}")
                       for k in range(KT)]
            pft2_sb = [pc.tile([128, 128], f16, tag=f"pft{t}", name=f"pft{t}")
                       for t in range(NT)]
            idh = pc.tile([128, 128], f16, tag="idh", name="idh")
            srp = pc.tile([128, 6], f32, tag="srp", name="srp")
            sxp = pc.tile([128, 6], f32, tag="sxp", name="sxp")

            sample_xts = {}

            def emit_xts_dma(s):
                xts = [psmp.tile([128, NP], f32r, tag=f"xt{k}", name=f"xt{k}",
                                 bufs=3) for k in range(KT)]
                for k in range(KT):
                    nc.sync.dma_start(xts[k][:], xT_d[s, k * 128:(k + 1) * 128, :])
                sample_xts[s] = xts

            xts0 = [psmp.tile([128, NP], f32r, tag=f"xt{k}", name=f"xt{k}",
                              bufs=3) for k in range(KT)]
            for k in range(KT):
                nc.sync.dma_start(win_sb[k][:], win_d[k * 128:(k + 1) * 128, :])
                nc.sync.dma_start(xts0[k][:], xT_d[0, k * 128:(k + 1) * 128, :])
            sample_xts[0] = xts0
            nc.sync.dma_start(idh[:], idh_d[:])
            for t in range(NT):
                nc.sync.dma_start(pft2_sb[t][:], pft2_d[t * 128:(t + 1) * 128, :])
            for k in range(KT):
                nc.sync.dma_start(wout_sb[k][:], wout_d[k * 128:(k + 1) * 128, :])
            nc.sync.dma_start(srp[:], srp_d[:])
            nc.sync.dma_start(sxp[:], sxp_d[:])

            cp = [0]  # alternating PSUM-evac engine (ACT / DVE)

            def evac(dst, src):
                cp[0] += 1
                if cp[0] % 2:
                    nc.scalar.copy(dst, src)
                else:
                    nc.vector.tensor_copy(dst, src)

            # per-sample persistent tiles
            sample_tiles = {}

            def get_sample(s):
                if s not in sample_tiles:
                    sample_tiles[s] = {
                        "inT": [psmp.tile([128, NP], f16, tag=f"inT{m}",
                                          name=f"inT{m}") for m in range(KT)],
                        "pnat": [psmp.tile([128, D], f16, tag=f"pn{t}",
                                           name=f"pn{t}") for t in range(NT)],
                        "rstk": psmp.tile([128, D], f16, tag="rstk", name="rstk"),
                        "dT": [psmp.tile([128, NP], f16, tag=f"dT{m}",
                                         name=f"dT{m}") for m in range(KT)],
                    }
                return sample_tiles[s]

            # ---------------- filler thunk builders ----------------

            def innerT_chunk(s, m, c0, c1):
                # inT[m][:, c0:c1] = (W_in^T x^T)[m-block, cols], fp16
                def th():
                    _mark(nc, f"inner{s}")
                    st = get_sample(s)
                    xts = sample_xts[s]
                    ps = pj1.tile([128, 512], f32, tag="proj", name="proj")
                    for k in range(KT):
                        nc.tensor.matmul(
                            ps[:, 0:c1 - c0],
                            win_sb[k][:, m * 128:(m + 1) * 128],
                            xts[k][:, c0:c1],
                            start=(k == 0), stop=(k == KT - 1),
                        )
                    evac(st["inT"][m][:, c0:c1], ps[:, 0:c1 - c0])
                return th

            def pnatT_group(s, t):
                # pnat[t] = transpose of inT token-block t (identity matmuls)
                def th():
                    _mark(nc, f"pnatT{s}")
                    st = get_sample(s)
                    pn = st["pnat"][t]
                    sz = 128 if t < NT - 1 else 16
                    if t == NT - 1:
                        nc.gpsimd.memset(pn[:], 0.0)
                    tp1 = ptp.tile([128, 512], f32, tag="tp", name="tp1")
                    tp2 = ptp.tile([128, 512], f32, tag="tp", name="tp2")
                    for hp in range(KT):
                        dst = tp1[0:sz, (hp % 4) * 128:(hp % 4) * 128 + 128] \
                            if hp < 4 else \
                            tp2[0:sz, (hp - 4) * 128:(hp - 4) * 128 + 128]
                        nc.tensor.matmul(
                            dst, st["inT"][hp][:, 1 + t * 128:1 + t * 128 + sz],
                            idh[:], start=True, stop=True,
                        )
                    evac(pn[0:sz, 0:512], tp1[0:sz, 0:512])
                    evac(pn[0:sz, 512:768], tp2[0:sz, 0:256])
                return th

            def pool_chunk(s, ci):
                # pooled landmark stack; stationary has duplicated columns so
                # rows 0:64 and 64:128 both hold the pooled matrix
                def th():
                    _mark(nc, f"pool{s}")
                    st = get_sample(s)
                    rstk = st["rstk"]
                    ps = pj1.tile([128, 512], f32, tag="proj", name="proj")
                    c0, c1 = (0, 512) if ci == 0 else (512, 768)
                    for t in range(NT):
                        nc.tensor.matmul(
                            ps[:, 0:c1 - c0], pft2_sb[t][:],
                            st["pnat"][t][:, c0:c1],
                            start=(t == 0), stop=(t == NT - 1),
                        )
                    if ci == 0:
                        evac(rstk[0:64, 0:512], ps[0:64, 0:512])
                        evac(rstk[64:128, 0:448], ps[64:128, 64:512])
                    else:
                        evac(rstk[0:64, 512:768], ps[0:64, 0:256])
                        evac(rstk[64:128, 448:704], ps[64:128, 0:256])
                return th

            def outproj_chunk(s, m, c0, c1):
                def th():
                    _mark(nc, f"out{s}")
                    st = get_sample(s)
                    xts = sample_xts[s]
                    ps = pj1.tile([128, 512], f32, tag="proj", name="proj")
                    for k in range(KT):
                        nc.tensor.matmul(
                            ps[:, 0:c1 - c0],
                            wout_sb[k][:, m * 128:(m + 1) * 128],
                            st["dT"][k][:, c0:c1],
                            start=(k == 0), stop=(k == KT - 1),
                        )
                    yst = pst.tile([128, 512], f32, tag="yst", name="yst")
                    xsl = xts[m][:, c0:c1].bitcast(f32)
                    cp[0] += 1
                    if cp[0] % 2:
                        nc.vector.tensor_add(
                            yst[:, 0:c1 - c0], ps[:, 0:c1 - c0], xsl)
                    else:
                        nc.scalar.copy(yst[:, 0:c1 - c0], ps[:, 0:c1 - c0])
                        nc.gpsimd.tensor_add(
                            yst[:, 0:c1 - c0], yst[:, 0:c1 - c0], xsl)
                    nc.sync.dma_start(
                        yT_d[s, m * 128:(m + 1) * 128, c0:c1],
                        yst[:, 0:c1 - c0])
                return th

            def prologue_thunks(s):
                th = []
                for m in range(KT):
                    for c0, c1 in ((0, 512), (512, NP)):
                        th.append(innerT_chunk(s, m, c0, c1))
                for t in range(NT):
                    th.append(pnatT_group(s, t))
                th.append(pool_chunk(s, 0))
                th.append(pool_chunk(s, 1))
                return th

            def outproj_thunks(s):
                return [outproj_chunk(s, m, c0, c1)
                        for m in range(KT) for c0, c1 in ((0, 512), (512, NP))]

            # ---------------- the attention pair chain ----------------

            def emit_pairs(s, filler):
                st = get_sample(s)
                rstk, inT, pnat, dT = (st["rstk"], st["inT"], st["pnat"],
                                       st["dT"])

                def fill():
                    if filler:
                        filler.pop(0)()

                ctx = {}

                def emitA(hp):
                    # G1..G4 + reps' update (through the DVE fused mul-add)
                    col = hp * 128  # reps col offset (both rstk halves)

                    # scratch psum: one bank, sliced per stage
                    sm = psm.tile([128, 512], f32, tag="sm", name="sm")

                    _mark(nc, f"p{s}.G1")
                    # G1: repsT via identity matmuls -> block-diag stationary
                    nc.tensor.matmul(sm[0:64, 0:64], rstk[0:64, col:col + 64],
                                     idh[0:64, 0:64], start=True, stop=True)
                    nc.tensor.matmul(sm[64:128, 64:128],
                                     rstk[64:128, col:col + 64],
                                     idh[64:128, 64:128], start=True, stop=True)
                    bd1 = ppr.tile([128, 128], f16, tag="bd1", name="bd1")
                    nc.gpsimd.memset(bd1[:], 0.0)
                    nc.scalar.copy(bd1[0:64, 0:64], sm[0:64, 0:64])
                    nc.vector.tensor_copy(bd1[64:128, 64:128],
                                          sm[64:128, 64:128])
                    fill()

                    _mark(nc, f"p{s}.G2")
                    # G2: logits for both heads in one stream (block-diag)
                    au = ppr.tile([128, NPAD], f16, tag="au", name="au")
                    nc.gpsimd.memset(au[:, NPATCH:NPAD], 0.0)
                    pl1 = pj2.tile([128, 512], f32, tag="proj", name="proj")
                    nc.tensor.matmul(pl1[:, 0:512], bd1[:], inT[hp][:, 1:513],
                                     start=True, stop=True)
                    pl2c = pj2.tile([128, 512], f32, tag="proj", name="proj")
                    nc.tensor.matmul(pl2c[:, 0:NPATCH - 512], bd1[:],
                                     inT[hp][:, 513:1 + NPATCH],
                                     start=True, stop=True)
                    nc.scalar.activation(au[:, 0:512], pl1[:, 0:512], AF.Exp,
                                         scale=SCALE)
                    nc.scalar.activation(au[:, 512:NPATCH],
                                         pl2c[:, 0:NPATCH - 512],
                                         AF.Exp, scale=SCALE)
                    den = ppr.tile([128, 1], f32, tag="den", name="den")
                    nc.vector.tensor_reduce(den[:], au[:, 0:NPATCH],
                                            mybir.AxisListType.XYZW,
                                            mybir.AluOpType.add)
                    rcp = ppr.tile([128, 1], f32, tag="rcp", name="rcp")
                    nc.vector.reciprocal(rcp[:], den[:])
                    fill()

                    _mark(nc, f"p{s}.G3")
                    # G3: attn_uT via identity matmuls, packed 4-to-a-bank
                    auT = ppr.tile([128, NPAD], f16, tag="auT", name="auT")
                    tp1 = ptp.tile([128, 512], f32, tag="tp", name="tp1")
                    tp2 = ptp.tile([128, 512], f32, tag="tp", name="tp2")
                    for t in range(NT):
                        dst = tp1[:, (t % 4) * 128:(t % 4) * 128 + 128] \
                            if t < 4 else tp2[:, (t - 4) * 128:(t - 4) * 128 + 128]
                        nc.tensor.matmul(dst, au[:, t * 128:(t + 1) * 128],
                                         idh[:], start=True, stop=True)
                    evac(auT[:, 0:512], tp1[:, 0:512])
                    evac(auT[:, 512:NPAD], tp2[:, 0:NPAD - 512])
                    fill()

                    _mark(nc, f"p{s}.G4")
                    # G4: rep_delta (pair-fused, diag blocks valid)
                    for t in range(NT):
                        nc.tensor.matmul(
                            sm[:, 128:256], auT[:, t * 128:(t + 1) * 128],
                            pnat[t][:, col:col + 128],
                            start=(t == 0), stop=(t == NT - 1),
                        )
                    svec = ppr.tile([128, 1], f32, tag="svec", name="svec")
                    nc.vector.tensor_scalar(
                        svec[:], rcp[:], srp[:, hp:hp + 1], None,
                        op0=mybir.AluOpType.mult)
                    # reps' = rep_delta * svec + reps, fused on DVE
                    ru = ppr.tile([128, DH], f16, tag="ru", name="ru")
                    nc.vector.scalar_tensor_tensor(
                        ru[0:64, :], sm[0:64, 128:192], svec[0:64, :],
                        rstk[0:64, col:col + 64],
                        op0=mybir.AluOpType.mult, op1=mybir.AluOpType.add)
                    nc.vector.scalar_tensor_tensor(
                        ru[64:128, :], sm[64:128, 192:256], svec[64:128, :],
                        rstk[64:128, col:col + 64],
                        op0=mybir.AluOpType.mult, op1=mybir.AluOpType.add)
                    fill()
                    ctx[hp] = (sm, au, ru, rcp)

                def emitB(hp):
                    sm, au, ru, rcp = ctx.pop(hp)

                    _mark(nc, f"p{s}.G5")
                    # G5: reps'^T (identity matmuls), stacked layout
                    nc.tensor.matmul(sm[0:64, 256:320], ru[0:64, :],
                                     idh[0:64, 0:64], start=True, stop=True)
                    nc.tensor.matmul(sm[64:128, 320:384], ru[64:128, :],
                                     idh[64:128, 64:128], start=True, stop=True)
                    ruT = ppr.tile([128, DH], f16, tag="ruT", name="ruT")
                    nc.scalar.copy(ruT[0:64, :], sm[0:64, 256:320])
                    nc.vector.tensor_copy(ruT[64:128, :], sm[64:128, 320:384])
                    fill()

                    _mark(nc, f"p{s}.G6")
                    # G6: attn2_u = exp(SCALE * reps' @ reps'^T)  (symmetric)
                    nc.tensor.matmul(sm[0:64, 384:448], ruT[0:64, :],
                                     ruT[0:64, :], start=True, stop=True)
                    nc.tensor.matmul(sm[64:128, 384:448], ruT[64:128, :],
                                     ruT[64:128, :], start=True, stop=True)
                    a2 = ppr.tile([128, DH], f16, tag="a2", name="a2")
                    d3 = ppr.tile([128, 1], f32, tag="d3", name="d3")
                    nc.scalar.activation(a2[:], sm[:, 384:448], AF.Exp,
                                         scale=SCALE)
                    nc.vector.tensor_reduce(d3[:], a2[:],
                                            mybir.AxisListType.XYZW,
                                            mybir.AluOpType.add)
                    rcp2 = ppr.tile([128, 1], f32, tag="rcp2", name="rcp2")
                    nc.vector.reciprocal(rcp2[:], d3[:])
                    fill()

                    _mark(nc, f"p{s}.G7")
                    # G7: rtr = attn2_u @ reps' (a2 symmetric -> use as lhsT)
                    nc.tensor.matmul(sm[0:64, 448:512], a2[0:64, :],
                                     ru[0:64, :], start=True, stop=True)
                    nc.tensor.matmul(sm[64:128, 448:512], a2[64:128, :],
                                     ru[64:128, :], start=True, stop=True)
                    s2 = ppr.tile([128, 1], f32, tag="s2", name="s2")
                    nc.vector.tensor_scalar(
                        s2[:], rcp2[:], rcp[:], sxp[:, hp:hp + 1],
                        op0=mybir.AluOpType.mult, op1=mybir.AluOpType.mult)
                    bd2 = ppr.tile([128, 128], f16, tag="bd2", name="bd2")
                    nc.gpsimd.memset(bd2[:], 0.0)
                    nc.vector.tensor_scalar(
                        bd2[0:64, 0:64], sm[0:64, 448:512], s2[0:64, :], None,
                        op0=mybir.AluOpType.mult)
                    nc.vector.tensor_scalar(
                        bd2[64:128, 64:128], sm[64:128, 448:512],
                        s2[64:128, :], None, op0=mybir.AluOpType.mult)
                    fill()

                    _mark(nc, f"p{s}.G8")
                    # G8: delta_tokens^T for both heads (block-diag stationary)
                    pd1 = pj2.tile([128, 512], f32, tag="proj", name="proj")
                    nc.tensor.matmul(pd1[:, 0:512], bd2[:], au[:, 0:512],
                                     start=True, stop=True)
                    pd2 = pj2.tile([128, 512], f32, tag="proj", name="proj")
                    nc.tensor.matmul(pd2[:, 0:NPATCH - 512], bd2[:],
                                     au[:, 512:NPATCH], start=True, stop=True)
                    evac(dT[hp][:, 1:513], pd1[:, 0:512])
                    evac(dT[hp][:, 513:1 + NPATCH], pd2[:, 0:NPATCH - 512])
                    nc.vector.tensor_copy(dT[hp][:, 0:1], inT[hp][:, 0:1])
                    fill()

                for hp in range(KT):
                    emitA(hp)
                    if hp > 0:
                        emitB(hp - 1)
                emitB(KT - 1)

            # ---------------- pipelined emission ----------------

            for th in prologue_thunks(0):
                th()
            for s in range(BS):
                filler = []
                if s + 1 < BS:
                    filler.append(lambda s=s: emit_xts_dma(s + 1))
                    filler += prologue_thunks(s + 1)
                if s > 0:
                    filler += outproj_thunks(s - 1)
                emit_pairs(s, filler)
                for th in filler:
                    th()
                # free the sample-tile dict entry (tiles rotate via tags)
                sample_tiles.pop(s - 1, None)
                sample_xts.pop(s - 1, None)
            for th in outproj_thunks(BS - 1):
                th()

    nc.finalize()
    return nc


def _get_state():
    if "nc" not in _STATE:
        _STATE["nc"] = _build_program()
    return _STATE["nc"]


def _host_inputs(x, W_in, W_out, step_x, step_rep):
    Ph = _pool_matrix(GRID, REP)
    Pfull = np.kron(Ph, Ph)                      # [64, 784]
    pft2 = np.zeros((NPAD, 128), np.float16)
    pft2[:NPATCH, 0:64] = Pfull.T.astype(np.float16)
    pft2[:NPATCH, 64:128] = Pfull.T.astype(np.float16)

    srp = np.zeros((128, 6), np.float32)
    sxp = np.zeros((128, 6), np.float32)
    sr = np.asarray(step_rep).reshape(HEADS)
    sx = np.asarray(step_x).reshape(HEADS)
    for hp in range(6):
        srp[0:64, hp] = sr[2 * hp]
        srp[64:128, hp] = sr[2 * hp + 1]
        sxp[0:64, hp] = sx[2 * hp]
        sxp[64:128, hp] = sx[2 * hp + 1]

    com = {
        "win": np.ascontiguousarray(W_in, np.float32),
        "wout": np.ascontiguousarray(W_out).astype(np.float16),
        "pft2": pft2,
        "idh": np.eye(128, dtype=np.float16),
        "srp": srp,
        "sxp": sxp,
    }
    xT = np.zeros((B, D, NP), np.float32)
    xT[:, :, 0:N] = np.transpose(x, (0, 2, 1))
    in_maps = []
    for c in range(NC):
        m = dict(com)
        m["xT"] = xT[c * BS:(c + 1) * BS]
        in_maps.append(m)
    return in_maps


def kernel(x, W_in, W_out, step_x, step_rep):
    from concourse import bass2jax
    nc = _get_state()
    in_maps = _host_inputs(x, W_in, W_out, step_x, step_rep)
    y = np.empty((B, N, D), np.float32)
    # The axon-tunneled result fetch has been observed to intermittently
    # return corrupted (NaN) buffers while the on-device result is fine;
    # re-run on a non-finite result.
    for _ in range(3):
        res = bass2jax.run_bass_via_pjrt(nc, in_maps, NC)
        for c in range(NC):
            y[c * BS:(c + 1) * BS] = np.transpose(
                res[c]["yT"][:, :, 0:N], (0, 2, 1))
        if np.isfinite(y).all():
            break
    return y
